# revision 30
# baseline (speedup 1.0000x reference)
"""AreaAttention Trainium2 kernel: B=8 data-parallel over 8 NeuronCores.

Reference computation (per sample, C=128 channels, N=H*W=4096 pixels):
    q = Wq@x + bq                    ('oc,bcn->bno' proper matmul)
    k = x * colsum(Wk) + bk          ('oc,bcn->bcn' keeps c: per-channel scale!)
    v = Wv@x + bv                    ('oc,bcn->bno')
    out = x + softmax(q^T k / sqrt(C)) @ v^T

Per-core design (one sample per core, no collectives):
  - q16/k16 stored [c, n] fp16; v16 stored [m, c] fp16 (PSUM accumulates fp32).
  - Scores computed TRANSPOSED: sT[m, n] = k_chunk^T @ q  (free dim 512).
  - exp(s/sqrt(C) - 4) on ScalarE, psum -> fp16 SBUF, paired into [128,2048]
    tiles (two m-chunks side by side) to halve DVE chain-add op count.
  - PV: out[c, n] += v_chunk^T @ expS  -> output directly in [c, n] layout.
  - Softmax denominator: 2 fp16 partial-sum chains over chunk-pairs, ones-matmul
    reduces partitions AND broadcasts row-sums, reciprocal, multiply, +residual.
  - Setup (projections) is interleaved into block 0's pair loop so the
    in-order engine streams reach the first exp fast.
"""
import numpy as np

C = 128
N = 4096          # 64*64
NB = 1024         # n-block span
NBLK = N // NB    # 4
MCH = N // C      # 32 m-chunks
NPAIR = MCH // 2  # 16 chunk-pairs per block
SCALE = 1.0 / np.sqrt(np.float32(C))
ESHIFT = -4.0

_cache = {}


def _build_nc():
    import concourse.tile as tile
    from concourse import bacc, mybir

    f32 = mybir.dt.float32
    f16 = mybir.dt.float16
    ADD = mybir.AluOpType.add
    MUL = mybir.AluOpType.mult
    EXP = mybir.ActivationFunctionType.Exp

    nc = bacc.Bacc("TRN2", target_bir_lowering=False)

    x_d = nc.dram_tensor("x", [C, N], f32, kind="ExternalInput")
    wqt16_d = nc.dram_tensor("wqt16", [C, C], f16, kind="ExternalInput")
    wks_d = nc.dram_tensor("wks", [C, 1], f32, kind="ExternalInput")
    wvt16_d = nc.dram_tensor("wvt16", [C, C], f16, kind="ExternalInput")
    bq_d = nc.dram_tensor("bq", [C, 1], f32, kind="ExternalInput")
    bk_d = nc.dram_tensor("bk", [C, 1], f32, kind="ExternalInput")
    bv_d = nc.dram_tensor("bv", [C, 1], f32, kind="ExternalInput")
    out_d = nc.dram_tensor("out", [C, N], f32, kind="ExternalOutput")

    with tile.TileContext(nc) as tc:
        with tc.tile_pool(name="big", bufs=1) as big, \
             tc.tile_pool(name="small", bufs=1) as small, \
             tc.tile_pool(name="es_pool", bufs=6) as es_pool, \
             tc.tile_pool(name="p_pool", bufs=4) as p_pool, \
             tc.tile_pool(name="work", bufs=2) as work, \
             tc.tile_pool(name="ps_sc", bufs=2, space="PSUM") as ps_sc, \
             tc.tile_pool(name="ps_pv", bufs=2, space="PSUM") as ps_pv:

            xfb = big.tile([C, N], f32, tag="xfb")      # x, then x + bv (residual)
            xf16 = big.tile([C, N], f16, tag="xf16")    # x fp16 (q/v proj, k build)
            q16 = big.tile([C, N], f16, tag="q16")
            k16 = big.tile([C, N], f16, tag="k16")
            v16 = big.tile([C, N], f16, tag="v16")      # chunk j at cols [128j,128j+128) = v[m, c]

            wqt16 = small.tile([C, C], f16, tag="wqt16")
            wks = small.tile([C, 1], f32, tag="wks")
            wvt16 = small.tile([C, C], f16, tag="wvt16")
            bq = small.tile([C, 1], f32, tag="bq")
            bk = small.tile([C, 1], f32, tag="bk")
            bv = small.tile([C, 1], f32, tag="bv")
            ebias = small.tile([C, 1], f32, tag="ebias")
            ones16 = small.tile([C, C], f16, tag="ones16")

            # x slices enqueue FIRST on the sync queue (they gate everything);
            # small weight DMAs go via the scalar HWDGE queue in parallel.
            for s in range(NBLK):
                sl = slice(s * NB, (s + 1) * NB)
                nc.sync.dma_start(xfb[:, sl], x_d[:, sl])
            nc.scalar.dma_start(wqt16[:], wqt16_d[:])
            nc.scalar.dma_start(wks[:], wks_d[:])
            nc.scalar.dma_start(wvt16[:], wvt16_d[:])
            nc.scalar.dma_start(bq[:], bq_d[:])
            nc.scalar.dma_start(bk[:], bk_d[:])
            nc.scalar.dma_start(bv[:], bv_d[:])
            nc.vector.memset(ebias[:], ESHIFT)
            nc.vector.memset(ones16[:], 1.0)

            def setup_qk(s):
                """xf16 cast, k16, q16 for one 1024-col slice."""
                sl = slice(s * NB, (s + 1) * NB)
                nc.vector.tensor_copy(xf16[:, sl], xfb[:, sl])
                nc.vector.tensor_scalar(k16[:, sl], xf16[:, sl], wks[:], bk[:],
                                        op0=MUL, op1=ADD)
                ps = ps_pv.tile([C, NB], f32, tag="pv", name=f"qps{s}")
                for h in range(2):
                    hsl = slice(s * NB + h * 512, s * NB + (h + 1) * 512)
                    nc.tensor.matmul(ps[:, h * 512:(h + 1) * 512], wqt16[:],
                                     xf16[:, hsl], start=True, stop=True)
                nc.vector.tensor_scalar(q16[:, sl], ps[:], bq[:], None, op0=ADD)

            def setup_v(s):
                """v16 chunks for one 1024-col slice (8 chunks, one psum tile)."""
                sl = slice(s * NB, (s + 1) * NB)
                psv = ps_pv.tile([C, NB], f32, tag="pv", name=f"vps{s}")
                for t in range(8):
                    j = s * 8 + t
                    nc.tensor.matmul(psv[:, t * C:(t + 1) * C],
                                     xf16[:, j * C:(j + 1) * C], wvt16[:],
                                     start=True, stop=True)
                nc.vector.tensor_copy(v16[:, sl], psv[:])

            def resid_slice(s):
                sl = slice(s * NB, (s + 1) * NB)
                nc.vector.tensor_scalar(xfb[:, sl], xfb[:, sl], bv[:], None, op0=ADD)

            def tail(nb, pv, chains):
                n0 = nb * NB
                last = nb == NBLK - 1
                # row-sum over partitions + both pair-halves; the ones[128,128]
                # stationary operand broadcasts the sum to all partitions.
                rs = ps_sc.tile([C, NB], f32, tag="sc", name=f"rs{nb}")
                for h in range(2):
                    hsl = slice(h * 512, (h + 1) * 512)
                    for ci in range(2):
                        nc.tensor.matmul(rs[:, hsl], ones16[:], chains[ci][:, hsl],
                                         start=(ci == 0), stop=False)
                        nc.tensor.matmul(rs[:, hsl], ones16[:],
                                         chains[ci][:, NB + h * 512:NB + (h + 1) * 512],
                                         start=False, stop=(ci == 1))
                # copy to SBUF so the slow reciprocal holds no PSUM bank.
                # For the final block ACT is idle, so it does the copy and the
                # normalize/DMA pipeline runs in 512-halves to drain sooner.
                rss = work.tile([C, NB], f32, tag="rss", name=f"rss{nb}")
                nc.vector.tensor_copy(rss[:], rs[:])
                rb = work.tile([C, NB], f32, tag="rb", name=f"rb{nb}")
                ep1 = work.tile([C, NB], f32, tag="ep1", name=f"ep{nb}")
                ost = work.tile([C, NB], f32, tag="ost", name=f"ost{nb}")
                halves = (slice(0, 512), slice(512, NB)) if last else (slice(0, NB),)
                for hs in halves:
                    osl = slice(n0 + hs.start, n0 + hs.stop)
                    nc.vector.reciprocal(rb[:, hs], rss[:, hs])
                    nc.vector.tensor_tensor(ep1[:, hs], pv[:, hs], rb[:, hs], op=MUL)
                    nc.vector.tensor_tensor(ost[:, hs], ep1[:, hs], xfb[:, osl], op=ADD)
                    nc.sync.dma_start(out_d[:, osl], ost[:, hs])

            # block 0 needs only slice-0 q/k before its first QK; v16 chunks
            # 0-7 are emitted between jp=0's exp and PV so they're off the
            # first-exp critical path. Remaining slices interleave into
            # block-0 pairs.
            setup_qk(0)

            for nb in range(NBLK):
                n0 = nb * NB
                pv = ps_pv.tile([C, NB], f32, tag="pv", name=f"pv{nb}")
                chains = [p_pool.tile([C, 2 * NB], f16, tag="pacc", name=f"pacc{nb}_{i}")
                          for i in range(2)]

                for jp in range(NPAIR):
                    if nb == 0 and jp in (3, 7, 11):
                        s = (jp + 1) // 4  # slices 1, 2, 3
                        setup_qk(s)
                        setup_v(s)
                    if nb == 0 and jp in (9, 10, 12, 13):
                        resid_slice(jp - 9 if jp < 11 else jp - 10)
                    es2 = es_pool.tile([C, 2 * NB], f16, tag="es", name=f"es{nb}_{jp}")
                    for u in range(2):
                        j = 2 * jp + u
                        ksl = slice(j * C, (j + 1) * C)
                        sc = ps_sc.tile([C, NB], f32, tag="sc", name=f"sc{nb}_{j}")
                        for h in range(2):
                            nc.tensor.matmul(sc[:, h * 512:(h + 1) * 512],
                                             k16[:, ksl],
                                             q16[:, n0 + h * 512:n0 + (h + 1) * 512],
                                             start=True, stop=True)
                        nc.scalar.activation(es2[:, u * NB:(u + 1) * NB], sc[:], EXP,
                                             bias=ebias[:], scale=float(SCALE))
                        if nb == 0 and jp == 0 and u == 0:
                            setup_v(0)
                        # PV: out[c,n] += v_chunk^T @ es
                        for h in range(2):
                            nc.tensor.matmul(pv[:, h * 512:(h + 1) * 512],
                                             v16[:, ksl],
                                             es2[:, u * NB + h * 512:u * NB + (h + 1) * 512],
                                             start=(j == 0), stop=(j == MCH - 1))
                    # denominator: 2 interleaved fp16 chains over chunk-pairs
                    ch = chains[jp % 2]
                    if jp < 2:
                        nc.vector.tensor_copy(ch[:], es2[:])
                    else:
                        nc.vector.tensor_tensor(ch[:], ch[:], es2[:], op=ADD)

                tail(nb, pv, chains)

    nc.finalize()
    return nc


def _get_nc():
    if "nc" not in _cache:
        _cache["nc"] = _build_nc()
    return _cache["nc"]


def make_in_maps(x, Wq, bq, Wk, bk, Wv, bv):
    x = np.asarray(x, dtype=np.float32)
    B = x.shape[0]
    wqt16 = np.ascontiguousarray(np.asarray(Wq, np.float32).T).astype(np.float16)
    wks = np.asarray(Wk, np.float32).sum(axis=0).reshape(C, 1)
    wvt16 = np.ascontiguousarray(np.asarray(Wv, np.float32).T).astype(np.float16)
    bq_ = np.asarray(bq, np.float32).reshape(C, 1)
    bk_ = np.asarray(bk, np.float32).reshape(C, 1)
    bv_ = np.asarray(bv, np.float32).reshape(C, 1)
    in_maps = []
    for i in range(B):
        in_maps.append({
            "x": np.ascontiguousarray(x[i].reshape(C, N)),
            "wqt16": wqt16, "wks": wks, "wvt16": wvt16,
            "bq": bq_, "bk": bk_, "bv": bv_,
        })
    return in_maps


def kernel(x, Wq, bq, Wk, bk, Wv, bv, _trace=False, _tmpdir=None):
    from concourse.bass_utils import run_bass_kernel_spmd

    x = np.asarray(x, dtype=np.float32)
    B, c, H, W = x.shape
    assert (c, H * W) == (C, N), (c, H, W)
    in_maps = make_in_maps(x, Wq, bq, Wk, bk, Wv, bv)
    nc = _get_nc()
    res = run_bass_kernel_spmd(nc, in_maps, core_ids=list(range(B)),
                               trace=_trace, tmpdir=_tmpdir)
    out = np.stack([res.results[i]["out"].reshape(C, H, W) for i in range(B)])
    if _trace:
        _cache["last_result"] = res
    return out.astype(np.float32)


# revision 40
# speedup vs baseline: 1.2150x; 1.2150x over previous
"""AreaAttention Trainium2 kernel: B=8 data-parallel over 8 NeuronCores.

Reference computation (per sample, C=128 channels, N=H*W=4096 pixels):
    q = Wq@x + bq                    ('oc,bcn->bno' proper matmul)
    k = x * colsum(Wk) + bk          ('oc,bcn->bcn' keeps c: per-channel scale!)
    v = Wv@x + bv                    ('oc,bcn->bno')
    out = x + softmax(q^T k / sqrt(C)) @ v^T

Per-core design (one sample per core, no collectives):
  - q16/k16 stored [c, n] fp16; v16 stored [m, c] fp16 (PSUM accumulates fp32).
  - Scores computed TRANSPOSED: sT[m, n] = k_chunk^T @ q  (free dim 512).
  - exp(s/sqrt(C) - 4) on ScalarE, psum -> fp16 SBUF, paired into [128,2048]
    tiles (two m-chunks side by side) to halve DVE chain-add op count.
  - PV: out[c, n] += v_chunk^T @ expS  -> output directly in [c, n] layout.
  - Softmax denominator: 2 fp16 partial-sum chains over chunk-pairs, ones-matmul
    reduces partitions AND broadcasts row-sums, reciprocal, multiply, +residual.
  - Setup (projections) is interleaved into block 0's pair loop so the
    in-order engine streams reach the first exp fast.
"""
import numpy as np

C = 128
N = 4096          # 64*64
NB = 1024         # n-block span
NBLK = N // NB    # 4
MCH = N // C      # 32 m-chunks
NPAIR = MCH // 2  # 16 chunk-pairs per block
SCALE = 1.0 / np.sqrt(np.float32(C))
ESHIFT = -4.0

_cache = {}


def _build_nc():
    import concourse.tile as tile
    from concourse import bacc, mybir

    f32 = mybir.dt.float32
    f16 = mybir.dt.float16
    ADD = mybir.AluOpType.add
    MUL = mybir.AluOpType.mult
    EXP = mybir.ActivationFunctionType.Exp

    nc = bacc.Bacc("TRN2", target_bir_lowering=False)

    x_d = nc.dram_tensor("x", [C, N], f32, kind="ExternalInput")
    wqt16_d = nc.dram_tensor("wqt16", [C, C], f16, kind="ExternalInput")
    wks_d = nc.dram_tensor("wks", [C, 1], f32, kind="ExternalInput")
    wvt16_d = nc.dram_tensor("wvt16", [C, C], f16, kind="ExternalInput")
    bq_d = nc.dram_tensor("bq", [C, 1], f32, kind="ExternalInput")
    bk_d = nc.dram_tensor("bk", [C, 1], f32, kind="ExternalInput")
    bv_d = nc.dram_tensor("bv", [C, 1], f32, kind="ExternalInput")
    out_d = nc.dram_tensor("out", [C, N], f32, kind="ExternalOutput")

    with tile.TileContext(nc) as tc:
        with tc.tile_pool(name="big", bufs=1) as big, \
             tc.tile_pool(name="small", bufs=1) as small, \
             tc.tile_pool(name="es_pool", bufs=8) as es_pool, \
             tc.tile_pool(name="p_pool", bufs=6) as p_pool, \
             tc.tile_pool(name="work", bufs=2) as work, \
             tc.tile_pool(name="ps_sc", bufs=2, space="PSUM") as ps_sc, \
             tc.tile_pool(name="ps_pv", bufs=2, space="PSUM") as ps_pv:

            xfb = big.tile([C, N], f32, tag="xfb")      # x, then x + bv (residual)
            xf16 = big.tile([C, N], f16, tag="xf16")    # x fp16 (q/v proj, k build)
            q16 = big.tile([C, N], f16, tag="q16")
            k16 = big.tile([C, N], f16, tag="k16")
            v16 = big.tile([C, N], f16, tag="v16")      # chunk j at cols [128j,128j+128) = v[m, c]

            wqt16 = small.tile([C, C], f16, tag="wqt16")
            wks = small.tile([C, 1], f32, tag="wks")
            wvt16 = small.tile([C, C], f16, tag="wvt16")
            bq = small.tile([C, 1], f32, tag="bq")
            bk = small.tile([C, 1], f32, tag="bk")
            bv = small.tile([C, 1], f32, tag="bv")
            ebias = small.tile([C, 1], f32, tag="ebias")
            ones16 = small.tile([C, C], f16, tag="ones16")

            # x slices enqueue FIRST on the sync queue (they gate everything);
            # slice 0 is split across the sync+scalar queues to land sooner.
            nc.sync.dma_start(xfb[:, 0:512], x_d[:, 0:512])
            nc.scalar.dma_start(xfb[:, 512:NB], x_d[:, 512:NB])
            for s in range(1, NBLK):
                sl = slice(s * NB, (s + 1) * NB)
                nc.sync.dma_start(xfb[:, sl], x_d[:, sl])
            nc.scalar.dma_start(wqt16[:], wqt16_d[:])
            nc.scalar.dma_start(wks[:], wks_d[:])
            nc.scalar.dma_start(wvt16[:], wvt16_d[:])
            nc.scalar.dma_start(bq[:], bq_d[:])
            nc.scalar.dma_start(bk[:], bk_d[:])
            nc.scalar.dma_start(bv[:], bv_d[:])
            nc.vector.memset(ebias[:], ESHIFT)
            nc.vector.memset(ones16[:], 1.0)

            def setup_qk(s):
                """xf16 cast, k16, q16 for one 1024-col slice."""
                sl = slice(s * NB, (s + 1) * NB)
                nc.vector.tensor_copy(xf16[:, sl], xfb[:, sl])
                nc.vector.tensor_scalar(k16[:, sl], xf16[:, sl], wks[:], bk[:],
                                        op0=MUL, op1=ADD)
                ps = ps_pv.tile([C, NB], f32, tag="pv", name=f"qps{s}")
                for h in range(2):
                    hsl = slice(s * NB + h * 512, s * NB + (h + 1) * 512)
                    nc.tensor.matmul(ps[:, h * 512:(h + 1) * 512], wqt16[:],
                                     xf16[:, hsl], start=True, stop=True)
                nc.vector.tensor_scalar(q16[:, sl], ps[:], bq[:], None, op0=ADD)

            def setup_v(s):
                """v16 chunks for one 1024-col slice (8 chunks, one psum tile)."""
                sl = slice(s * NB, (s + 1) * NB)
                psv = ps_pv.tile([C, NB], f32, tag="pv", name=f"vps{s}")
                for t in range(8):
                    j = s * 8 + t
                    nc.tensor.matmul(psv[:, t * C:(t + 1) * C],
                                     xf16[:, j * C:(j + 1) * C], wvt16[:],
                                     start=True, stop=True)
                nc.vector.tensor_copy(v16[:, sl], psv[:])

            def resid_slice(s):
                sl = slice(s * NB, (s + 1) * NB)
                nc.vector.tensor_scalar(xfb[:, sl], xfb[:, sl], bv[:], None, op0=ADD)

            def tail(nb, pv, chains, last_es2):
                n0 = nb * NB
                last = nb == NBLK - 1
                # row-sum over partitions; the ones[128,128] stationary operand
                # broadcasts the sum to all partitions. Sources: the two chains
                # (ready one pair early) + the final pair's es2 directly.
                rs = ps_sc.tile([C, NB], f32, tag="sc", name=f"rs{nb}")
                srcs = [(chains[0], slice(0, NB)), (chains[1], slice(0, NB))]
                if last_es2 is not None:
                    srcs.append((last_es2, slice(0, NB)))
                for h in range(2):
                    hsl = slice(h * 512, (h + 1) * 512)
                    for ci, (src, _) in enumerate(srcs):
                        nc.tensor.matmul(rs[:, hsl], ones16[:], src[:, hsl],
                                         start=(ci == 0), stop=False)
                        nc.tensor.matmul(rs[:, hsl], ones16[:],
                                         src[:, NB + h * 512:NB + (h + 1) * 512],
                                         start=False, stop=(ci == len(srcs) - 1))
                # copy to SBUF (in halves) so the slow reciprocal holds no PSUM
                # bank; the normalize/DMA pipeline runs per 512-half.
                rss = work.tile([C, NB], f32, tag="rss", name=f"rss{nb}")
                rb = work.tile([C, NB], f32, tag="rb", name=f"rb{nb}")
                ep1 = work.tile([C, NB], f32, tag="ep1", name=f"ep{nb}")
                ost = work.tile([C, NB], f32, tag="ost", name=f"ost{nb}")
                halves = (slice(0, 512), slice(512, NB)) if last else (slice(0, NB),)
                for hs in halves:
                    osl = slice(n0 + hs.start, n0 + hs.stop)
                    nc.vector.tensor_copy(rss[:, hs], rs[:, hs])
                    nc.vector.reciprocal(rb[:, hs], rss[:, hs])
                    nc.vector.tensor_tensor(ep1[:, hs], pv[:, hs], rb[:, hs], op=MUL)
                    nc.vector.tensor_tensor(ost[:, hs], ep1[:, hs], xfb[:, osl], op=ADD)
                    nc.sync.dma_start(out_d[:, osl], ost[:, hs])

            # block 0 needs only slice-0 q/k before its first QK; v16 chunks
            # 0-7 are emitted between jp=0's exp and PV so they're off the
            # first-exp critical path. Remaining slices interleave into
            # block-0 pairs.
            setup_qk(0)

            def emit_qk_exp(nb, jp):
                """QK matmuls + exp for one chunk-pair; returns the es2 tile."""
                n0 = nb * NB
                es2 = es_pool.tile([C, 2 * NB], f16, tag="es", name=f"es{nb}_{jp}")
                for u in range(2):
                    j = 2 * jp + u
                    ksl = slice(j * C, (j + 1) * C)
                    sc = ps_sc.tile([C, NB], f32, tag="sc", name=f"sc{nb}_{j}")
                    for h in range(2):
                        nc.tensor.matmul(sc[:, h * 512:(h + 1) * 512],
                                         k16[:, ksl],
                                         q16[:, n0 + h * 512:n0 + (h + 1) * 512],
                                         start=True, stop=True)
                    nc.scalar.activation(es2[:, u * NB:(u + 1) * NB], sc[:], EXP,
                                         bias=ebias[:], scale=float(SCALE))
                return es2

            def emit_pv(nb, jp, pv, es2):
                for u in range(2):
                    j = 2 * jp + u
                    ksl = slice(j * C, (j + 1) * C)
                    for h in range(2):
                        nc.tensor.matmul(pv[:, h * 512:(h + 1) * 512],
                                         v16[:, ksl],
                                         es2[:, u * NB + h * 512:u * NB + (h + 1) * 512],
                                         start=(j == 0), stop=(j == MCH - 1))

            NHOIST = 2  # next-block QK/exp pairs emitted ahead of each tail
            hoisted = {}
            for nb in range(NBLK):
                pv = ps_pv.tile([C, NB], f32, tag="pv", name=f"pv{nb}")
                chains = [p_pool.tile([C, 2 * NB], f16, tag="pacc", name=f"pacc{nb}_{i}")
                          for i in range(2)]

                for jp in range(NPAIR):
                    if nb == 0 and jp in (3, 7, 11):
                        s = (jp + 1) // 4  # slices 1, 2, 3
                        setup_qk(s)
                        setup_v(s)
                    if nb == 0 and jp in (9, 10, 12, 13):
                        resid_slice(jp - 9 if jp < 11 else jp - 10)
                    es2 = hoisted.pop((nb, jp), None)
                    if es2 is None:
                        es2 = emit_qk_exp(nb, jp)
                    if nb == 0 and jp == 0:
                        setup_v(0)
                    # hoist the NEXT block's first QK/exp pairs ahead of this
                    # block's final PVs + tail so ACT never waits at the boundary
                    if jp == NPAIR - 1 and nb < NBLK - 1:
                        for hj in range(NHOIST):
                            hoisted[(nb + 1, hj)] = emit_qk_exp(nb + 1, hj)
                    emit_pv(nb, jp, pv, es2)
                    # denominator: 2 interleaved fp16 chains over chunk-pairs.
                    # In the FINAL block the last pair skips the chain -- its
                    # contribution goes straight into the rowsum matmuls.
                    if jp == NPAIR - 1 and nb == NBLK - 1:
                        last_es2 = es2
                    elif jp < 2:
                        nc.vector.tensor_copy(chains[jp % 2][:], es2[:])
                    else:
                        ch = chains[jp % 2]
                        nc.vector.tensor_tensor(ch[:], ch[:], es2[:], op=ADD)

                tail(nb, pv, chains, last_es2 if nb == NBLK - 1 else None)

    nc.finalize()
    return nc


def _get_nc():
    if "nc" not in _cache:
        _cache["nc"] = _build_nc()
    return _cache["nc"]


def make_in_maps(x, Wq, bq, Wk, bk, Wv, bv):
    x = np.asarray(x, dtype=np.float32)
    B = x.shape[0]
    wqt16 = np.ascontiguousarray(np.asarray(Wq, np.float32).T).astype(np.float16)
    wks = np.asarray(Wk, np.float32).sum(axis=0).reshape(C, 1)
    wvt16 = np.ascontiguousarray(np.asarray(Wv, np.float32).T).astype(np.float16)
    bq_ = np.asarray(bq, np.float32).reshape(C, 1)
    bk_ = np.asarray(bk, np.float32).reshape(C, 1)
    bv_ = np.asarray(bv, np.float32).reshape(C, 1)
    in_maps = []
    for i in range(B):
        in_maps.append({
            "x": np.ascontiguousarray(x[i].reshape(C, N)),
            "wqt16": wqt16, "wks": wks, "wvt16": wvt16,
            "bq": bq_, "bk": bk_, "bv": bv_,
        })
    return in_maps


def kernel(x, Wq, bq, Wk, bk, Wv, bv, _trace=False, _tmpdir=None):
    from concourse.bass_utils import run_bass_kernel_spmd

    x = np.asarray(x, dtype=np.float32)
    B, c, H, W = x.shape
    assert (c, H * W) == (C, N), (c, H, W)
    in_maps = make_in_maps(x, Wq, bq, Wk, bk, Wv, bv)
    nc = _get_nc()
    res = run_bass_kernel_spmd(nc, in_maps, core_ids=list(range(B)),
                               trace=_trace, tmpdir=_tmpdir)
    out = np.stack([res.results[i]["out"].reshape(C, H, W) for i in range(B)])
    if _trace:
        _cache["last_result"] = res
    return out.astype(np.float32)
